# revision 37
# baseline (speedup 1.0000x reference)
"""Non-local block (no softmax) on 8 Trainium2 cores, data-parallel over batch.

Math: per sample X [N=4096, C=256] (N = 64*64 spatial, C channels):
    theta = X Wt, phi = X Wp, g = X Wg          (biases are zero)
    y = (theta phi^T / N) g  ->  associativity (no softmax):
      y = X L G R,   L = Wt Wp^T,  R = Wg (Ww*s) / N,  G = X^T X
    z = y + t2 + X,  s = gamma*rsqrt(var+eps),  t2 = (b_W - mean)*s + beta

Device computes delta^T = (L G R)^T X^T + t2 in bf16; host adds X (f32).
G is computed from an fp8e4 copy of X (error only enters the delta term,
measured ~5.7e-3 rel overall); the z matmuls use a host-transposed bf16
X^T, so no PE transposes at all (they cost ~275ns each and poison HAM
warmth). Chain M2 = L (G R) uses G's symmetry: two small bf16 GEMMs.
One sample per NeuronCore.
"""

import numpy as np
import ml_dtypes

B, H, W, C = 8, 64, 64, 256
IC = C // 2
N = H * W
NCHUNK = N // 128  # 32
BN_EPS = 1e-3

_CACHE = {}
DEFAULT_MODE = "v1"


def _build_nc(mode: str):
    import concourse.bacc as bacc
    import concourse.mybir as mybir
    import concourse.tile as tile

    F32 = mybir.dt.float32
    BF16 = mybir.dt.bfloat16
    FP8 = mybir.dt.float8e4

    nc = bacc.Bacc("TRN2", target_bir_lowering=False, debug=False)

    # x packed on host so SBUF chunk t=[n-rows t*128..] sits at cols t*256
    x8_d = nc.dram_tensor("x8", [128, 32 * 256], FP8, kind="ExternalInput")
    xt_d = nc.dram_tensor("xt", [C, N], BF16, kind="ExternalInput")
    wl_d = nc.dram_tensor("wl", [128, 1024], BF16, kind="ExternalInput")
    t2c_d = nc.dram_tensor("t2c", [128, 2], F32, kind="ExternalInput")
    dt_d = nc.dram_tensor("dt", [C, N], BF16, kind="ExternalOutput")

    with tile.TileContext(nc) as tc:
        with (
            tc.tile_pool(name="const", bufs=1) as cpool,
            tc.tile_pool(name="big", bufs=1) as bpool,
            tc.tile_pool(name="psg", bufs=1, space="PSUM") as psg,
            tc.tile_pool(name="psz", bufs=3, space="PSUM") as psz,
        ):
            wl = cpool.tile([128, 1024], BF16, tag="wl")
            t2c = cpool.tile([128, 2], F32, tag="t2c")
            wz = cpool.tile([128, 512], BF16, tag="wz")
            nc.vector.memset(wz[:], 0.0)

            # x8 pieces alternate between the two DMA rings in consumption
            # order: each ring transfers its next piece while the other's
            # is being consumed, halving effective input latency
            XSPLIT = [(0, 2, 0), (2, 2, 1), (4, 2, 0), (6, 2, 1),
                      (8, 4, 0), (12, 4, 1), (16, 8, 0), (24, 8, 1)]
            x8_t = [bpool.tile([128, 256 * n], FP8, tag=f"x8_{u}",
                               name=f"x8_{u}")
                    for u, (c0, n, r) in enumerate(XSPLIT)]
            # xt quarters: [k-half of C] x [half of N], 512KB each
            xt_t = [[bpool.tile([128, 2048], BF16, tag=f"xt{k}_{h}",
                                name=f"xt{k}_{h}") for h in range(2)]
                    for k in range(2)]

            # ---- input DMAs in consumption order, alternating rings
            for u, (c0, n, r) in enumerate(XSPLIT):
                (nc.sync if r == 0 else nc.scalar).dma_start(
                    x8_t[u][:], x8_d[:, c0 * 256:(c0 + n) * 256])
            nc.sync.dma_start(wl[:], wl_d[:])
            nc.sync.dma_start(t2c[:], t2c_d[:])
            for k in range(2):
                for h in range(2):
                    (nc.sync if k == 0 else nc.scalar).dma_start(
                        xt_t[k][h][:],
                        xt_d[k * 128:(k + 1) * 128, h * 2048:(h + 1) * 2048])

            # dummies bridge PE from preamble end to the first x8 piece
            # landing (~2.5us) so HAM warmth builds with no idle gap
            wu = psz.tile([128, 1024], F32, tag="z")
            for _ in range(4):
                nc.tensor.matmul(wu[:, 0:512], wz[:, 0:128], wz[:],
                                 start=True, stop=True, skip_group_check=True)

            # ---- phase 1: G = X^T X (fp8, streams behind the x8 DMAs),
            # split into halves A (t<16) and B so the A-half of the chain
            # (copy + S accumulation) hides under the B-half matmuls
            # each half's (rows 0-127, rows 128-255) pair shares one PSUM
            # bank: the first matmul's start=True clears has_written for the
            # whole bank, so the second group opens with start=False
            gps = [psg.tile([128, 512], F32, tag=f"gp{ab}", name=f"gp{ab}")
                   for ab in range(2)]
            g_s = [[bpool.tile([128, 256], BF16, tag=f"g_s{j}{ab}",
                               name=f"g_s{j}{ab}") for j in range(2)]
                   for ab in range(2)]
            s_s = bpool.tile([128, 512], BF16, tag="s_s")
            ps = psz.tile([128, 1024], F32, tag="z")

            def g_mms(t):
                u = next(i for i, (c0, n, r) in enumerate(XSPLIT)
                         if c0 <= t < c0 + n)
                xn = x8_t[u]
                o = (t - XSPLIT[u][0]) * 256
                xc = xn[:, o:o + 256]
                ab = t // 16
                nc.tensor.matmul(gps[ab][:, 0:256], xn[:, o:o + 128], xc,
                                 start=(t % 16 == 0), stop=(t % 16 == 15),
                                 skip_group_check=True)
                nc.tensor.matmul(gps[ab][:, 256:512], xn[:, o + 128:o + 256],
                                 xc, start=False, stop=(t % 16 == 15),
                                 skip_group_check=True)

            def s_mms(ab):
                for i in range(2):
                    for k in range(2):
                        nc.tensor.matmul(ps[:, i * 512:i * 512 + 256],
                                         g_s[ab][k][:, i * 128:(i + 1) * 128],
                                         wl[:, k * 256:(k + 1) * 256],
                                         start=(ab == 0 and k == 0),
                                         stop=(ab == 1 and k == 1))

            for t in range(16):
                g_mms(t)
            nc.vector.tensor_copy(g_s[0][0][:], gps[0][:, 0:256])
            nc.scalar.copy(g_s[0][1][:], gps[0][:, 256:512])
            for t in range(16, 24):
                g_mms(t)
            s_mms(0)  # hides under the B-half of G
            for t in range(24, NCHUNK):
                g_mms(t)

            # ---- phase 2 tail: finish S, then M2 = L S (bf16)
            nc.vector.tensor_copy(g_s[1][0][:], gps[1][:, 0:256])
            nc.scalar.copy(g_s[1][1][:], gps[1][:, 256:512])
            s_mms(1)
            # filler matmuls run while the copies drain, keeping HAM warm
            for _ in range(4):
                nc.tensor.matmul(gps[0][:, 0:256], wz[:, 0:128], wz[:, 0:256],
                                 start=True, stop=True, skip_group_check=True)
            nc.vector.tensor_copy(s_s[:, 0:256], ps[:, 0:256])
            nc.scalar.copy(s_s[:, 256:512], ps[:, 512:768])

            # M2 = L S: row-block m (= lhsT k-chunk for phase 3) per bank
            m2_s = bpool.tile([128, 512], BF16, tag="m2_s")
            pm = psz.tile([128, 1024], F32, tag="z")
            for m in range(2):
                for k in range(2):
                    nc.tensor.matmul(
                        pm[:, m * 512:m * 512 + 256],
                        wl[:, 512 + k * 256 + m * 128:512 + k * 256 + m * 128 + 128],
                        s_s[:, k * 256:(k + 1) * 256],
                        start=(k == 0), stop=(k == 1))
            for _ in range(4):
                nc.tensor.matmul(gps[0][:, 256:512], wz[:, 0:128],
                                 wz[:, 0:256],
                                 start=True, stop=True, skip_group_check=True)
            nc.vector.tensor_copy(m2_s[:, 0:256], pm[:, 0:256])
            nc.scalar.copy(m2_s[:, 256:512], pm[:, 512:768])

            # ---- phase 3: delta^T = M2^T X^T + t2 (bf16), stream out
            # pz spans 2 PSUM banks; 2 col-blocks of 512 per drain/store
            z_s = [[bpool.tile([128, 1024], BF16, tag=f"z{m}_{q}",
                               name=f"z{m}_{q}") for q in range(4)]
                   for m in range(2)]
            for q in range(4):
                h, p2 = q // 2, q % 2
                for m in range(2):
                    pz = psz.tile([128, 1024], F32, tag="z")
                    for j in range(2):
                        co = (p2 * 2 + j) * 512
                        nc.tensor.matmul(
                            pz[:, j * 512:(j + 1) * 512],
                            m2_s[:, m * 128:(m + 1) * 128],
                            xt_t[0][h][:, co:co + 512],
                            start=True, stop=False)
                        nc.tensor.matmul(
                            pz[:, j * 512:(j + 1) * 512],
                            m2_s[:, 256 + m * 128:256 + (m + 1) * 128],
                            xt_t[1][h][:, co:co + 512],
                            start=False, stop=True)
                    dst = z_s[m][q]
                    if q == 3:
                        # final quarter: split drains for minimum latency
                        nc.vector.tensor_scalar_add(dst[:, 0:512],
                                                    pz[:, 0:512],
                                                    t2c[:, m:m + 1])
                        nc.scalar.activation(
                            dst[:, 512:1024], pz[:, 512:1024],
                            mybir.ActivationFunctionType.Identity,
                            bias=t2c[:, m:m + 1])
                    elif (q * 2 + m) % 2 == 0:
                        nc.vector.tensor_scalar_add(dst[:], pz[:],
                                                    t2c[:, m:m + 1])
                    else:
                        nc.scalar.activation(
                            dst[:], pz[:],
                            mybir.ActivationFunctionType.Identity,
                            bias=t2c[:, m:m + 1])
                    if q == 3:
                        # final stores split across both rings for a short tail
                        nc.scalar.dma_start(
                            dt_d[m * 128:(m + 1) * 128,
                                 q * 1024:q * 1024 + 512], dst[:, 0:512])
                        nc.sync.dma_start(
                            dt_d[m * 128:(m + 1) * 128,
                                 q * 1024 + 512:(q + 1) * 1024],
                            dst[:, 512:1024])
                    else:
                        nc.sync.dma_start(
                            dt_d[m * 128:(m + 1) * 128,
                                 q * 1024:(q + 1) * 1024],
                            dst[:])

    nc.compile()
    return nc


def _get_nc(mode=DEFAULT_MODE):
    key = ("nc", mode)
    if key not in _CACHE:
        _CACHE[key] = _build_nc(mode)
    return _CACHE[key]


def _fold_params(w_g, b_g, w_theta, b_theta, w_phi, b_phi, w_W, b_W,
                 bn_gamma, bn_beta, bn_mean, bn_var):
    f32 = np.float32
    bf = ml_dtypes.bfloat16
    s = (bn_gamma / np.sqrt(bn_var + BN_EPS)).astype(f32)
    t2 = ((b_W - bn_mean) * s + bn_beta).astype(f32)
    L = (np.asarray(w_theta, f32) @ np.asarray(w_phi, f32).T).astype(f32)
    R = (np.asarray(w_g, f32) @ (np.asarray(w_W, f32) * s[None, :]) / N).astype(f32)
    # wl[:, :512][p, k*256+j] = R[k*128+p, j]; wl[:, 512:] likewise for L^T
    pack = lambda M: M.reshape(2, 128, 256).transpose(1, 0, 2).reshape(128, 512)
    wl = np.ascontiguousarray(
        np.concatenate([pack(R), pack(np.ascontiguousarray(L.T))], axis=1)
        .astype(bf))
    t2c = np.ascontiguousarray(t2.reshape(2, 128).T, dtype=f32)
    return wl, t2c


def _reference_fallback(x, w_g, b_g, w_theta, b_theta, w_phi, b_phi, w_W, b_W,
                        bn_gamma, bn_beta, bn_mean, bn_var):
    b, h, w, c = x.shape
    n = h * w
    xf = x.reshape(b, n, c).astype(np.float32)
    g_x = xf @ w_g + b_g
    theta_x = xf @ w_theta + b_theta
    phi_x = xf @ w_phi + b_phi
    a = np.einsum("bnd,bne->bde", phi_x, g_x) / n
    y = theta_x @ a
    w_y = y @ w_W + b_W
    w_y = bn_gamma * (w_y - bn_mean) / np.sqrt(bn_var + BN_EPS) + bn_beta
    return (w_y.reshape(b, h, w, c) + x).astype(np.float32)


def run_sharded(x, folded, mode=DEFAULT_MODE, trace=False):
    from concourse.bass_utils import run_bass_kernel_spmd

    nc = _get_nc(mode)
    wl, t2c = folded
    bf = ml_dtypes.bfloat16
    f8 = ml_dtypes.float8_e4m3
    xf = np.asarray(x, dtype=np.float32).reshape(B, N, C)
    # x8[p, t*256+c] = x[t*128+p, c]
    x8 = np.ascontiguousarray(
        np.clip(xf, -240.0, 240.0).reshape(B, 32, 128, 256)
        .transpose(0, 2, 1, 3).reshape(B, 128, 8192).astype(f8))
    xt = np.ascontiguousarray(xf.transpose(0, 2, 1).astype(bf))
    in_maps = [
        {"x8": x8[i], "xt": xt[i], "wl": wl, "t2c": t2c}
        for i in range(B)
    ]
    res = run_bass_kernel_spmd(nc, in_maps, list(range(B)), trace=trace)
    z = xf + np.stack(
        [np.asarray(res.results[i]["dt"], np.float32).T for i in range(B)],
        axis=0)
    return np.ascontiguousarray(z.reshape(B, H, W, C)), res


def kernel(x, w_g, b_g, w_theta, b_theta, w_phi, b_phi, w_W, b_W,
           bn_gamma, bn_beta, bn_mean, bn_var):
    args = dict(w_g=np.asarray(w_g), b_g=np.asarray(b_g),
                w_theta=np.asarray(w_theta), b_theta=np.asarray(b_theta),
                w_phi=np.asarray(w_phi), b_phi=np.asarray(b_phi),
                w_W=np.asarray(w_W), b_W=np.asarray(b_W),
                bn_gamma=np.asarray(bn_gamma), bn_beta=np.asarray(bn_beta),
                bn_mean=np.asarray(bn_mean), bn_var=np.asarray(bn_var))
    x = np.asarray(x)
    # the device path folds the (zero) projection biases away; anything else
    # (never produced by setup_inputs) gets the exact host fallback
    if (np.any(args["b_g"]) or np.any(args["b_theta"]) or np.any(args["b_phi"])
            or x.shape != (B, H, W, C)):
        return _reference_fallback(x, **{k: v for k, v in args.items()})
    folded = _fold_params(**args)
    z, _ = run_sharded(x, folded)
    return z


# revision 38
# speedup vs baseline: 1.0596x; 1.0596x over previous
"""Non-local block (no softmax) on 8 Trainium2 cores, data-parallel over batch.

Math: per sample X [N=4096, C=256] (N = 64*64 spatial, C channels):
    theta = X Wt, phi = X Wp, g = X Wg          (biases are zero)
    y = (theta phi^T / N) g  ->  associativity (no softmax):
      y = X L G R,   L = Wt Wp^T,  R = Wg (Ww*s) / N,  G = X^T X
    z = y + t2 + X,  s = gamma*rsqrt(var+eps),  t2 = (b_W - mean)*s + beta

Device computes delta^T = (L G R)^T X^T + t2 in bf16; host adds X (f32).
G is computed from an fp8e4 copy of X (error only enters the delta term,
measured ~5.7e-3 rel overall); the z matmuls use a host-transposed bf16
X^T, so no PE transposes at all (they cost ~275ns each and poison HAM
warmth). Chain M2 = L (G R) uses G's symmetry: two small bf16 GEMMs.
One sample per NeuronCore.
"""

import numpy as np
import ml_dtypes

B, H, W, C = 8, 64, 64, 256
IC = C // 2
N = H * W
NCHUNK = N // 128  # 32
BN_EPS = 1e-3

_CACHE = {}
DEFAULT_MODE = "v1"


def _build_nc(mode: str):
    import concourse.bacc as bacc
    import concourse.mybir as mybir
    import concourse.tile as tile

    F32 = mybir.dt.float32
    BF16 = mybir.dt.bfloat16
    FP8 = mybir.dt.float8e4

    nc = bacc.Bacc("TRN2", target_bir_lowering=False, debug=False)

    # x packed on host so SBUF chunk t=[n-rows t*128..] sits at cols t*256
    x8_d = nc.dram_tensor("x8", [128, 32 * 256], FP8, kind="ExternalInput")
    xt_d = nc.dram_tensor("xt", [C, N], BF16, kind="ExternalInput")
    wl_d = nc.dram_tensor("wl", [128, 1024], BF16, kind="ExternalInput")
    t2c_d = nc.dram_tensor("t2c", [128, 2], F32, kind="ExternalInput")
    dt_d = nc.dram_tensor("dt", [C, N], BF16, kind="ExternalOutput")

    with tile.TileContext(nc) as tc:
        with (
            tc.tile_pool(name="const", bufs=1) as cpool,
            tc.tile_pool(name="big", bufs=1) as bpool,
            tc.tile_pool(name="psg", bufs=1, space="PSUM") as psg,
            tc.tile_pool(name="psz", bufs=3, space="PSUM") as psz,
        ):
            wl = cpool.tile([128, 1024], BF16, tag="wl")
            t2c = cpool.tile([128, 2], F32, tag="t2c")
            wz = cpool.tile([128, 512], BF16, tag="wz")
            nc.vector.memset(wz[:], 0.0)

            # x8 pieces alternate between the two DMA rings in consumption
            # order: each ring transfers its next piece while the other's
            # is being consumed, halving effective input latency
            XSPLIT = [(0, 2, 0), (2, 2, 1), (4, 2, 0), (6, 2, 1),
                      (8, 4, 0), (12, 4, 1), (16, 8, 0), (24, 8, 1)]
            x8_t = [bpool.tile([128, 256 * n], FP8, tag=f"x8_{u}",
                               name=f"x8_{u}")
                    for u, (c0, n, r) in enumerate(XSPLIT)]
            # xt quarters: [k-half of C] x [half of N], 512KB each
            xt_t = [[bpool.tile([128, 2048], BF16, tag=f"xt{k}_{h}",
                                name=f"xt{k}_{h}") for h in range(2)]
                    for k in range(2)]

            # ---- input DMAs in consumption order, alternating rings
            for u, (c0, n, r) in enumerate(XSPLIT):
                (nc.sync if r == 0 else nc.scalar).dma_start(
                    x8_t[u][:], x8_d[:, c0 * 256:(c0 + n) * 256])
            nc.sync.dma_start(wl[:], wl_d[:])
            nc.sync.dma_start(t2c[:], t2c_d[:])
            for k in range(2):
                for h in range(2):
                    (nc.sync if k == 0 else nc.scalar).dma_start(
                        xt_t[k][h][:],
                        xt_d[k * 128:(k + 1) * 128, h * 2048:(h + 1) * 2048])

            # dummies bridge PE from preamble end to the first x8 piece
            # landing (~2.5us) so HAM warmth builds with no idle gap
            wu = psz.tile([128, 1024], F32, tag="z")
            for _ in range(4):
                nc.tensor.matmul(wu[:, 0:512], wz[:, 0:128], wz[:],
                                 start=True, stop=True, skip_group_check=True)

            # ---- phase 1: G = X^T X (fp8, streams behind the x8 DMAs).
            # G's two row-blocks share one PSUM bank: the first matmul's
            # start=True clears has_written for the whole bank, so the
            # second group opens with start=False (verified on HW)
            gps = psg.tile([128, 512], F32, tag="gp")
            g_s = [bpool.tile([128, 256], BF16, tag=f"g_s{j}", name=f"g_s{j}")
                   for j in range(2)]
            for t in range(NCHUNK):
                u = next(i for i, (c0, n, r) in enumerate(XSPLIT)
                         if c0 <= t < c0 + n)
                xn = x8_t[u]
                o = (t - XSPLIT[u][0]) * 256
                xc = xn[:, o:o + 256]
                nc.tensor.matmul(gps[:, 0:256], xn[:, o:o + 128], xc,
                                 start=(t == 0), stop=(t == NCHUNK - 1),
                                 skip_group_check=True)
                nc.tensor.matmul(gps[:, 256:512], xn[:, o + 128:o + 256],
                                 xc, start=False, stop=(t == NCHUNK - 1),
                                 skip_group_check=True)

            # ---- phase 2: M2 = L (G R) in bf16; S and M2 share one psz slot
            nc.vector.tensor_copy(g_s[0][:], gps[:, 0:256])
            nc.scalar.copy(g_s[1][:], gps[:, 256:512])
            s_s = bpool.tile([128, 512], BF16, tag="s_s")
            psm = psz.tile([128, 1024], F32, tag="z")
            for k in range(2):
                for i in range(2):
                    nc.tensor.matmul(psm[:, i * 512:i * 512 + 256],
                                     g_s[k][:, i * 128:(i + 1) * 128],
                                     wl[:, k * 256:(k + 1) * 256],
                                     start=(k == 0), stop=(k == 1))
            # filler matmuls run while the copies drain, keeping HAM warm
            for _ in range(4):
                nc.tensor.matmul(gps[:, 0:256], wz[:, 0:128], wz[:, 0:256],
                                 start=True, stop=True, skip_group_check=True)
            nc.vector.tensor_copy(s_s[:, 0:256], psm[:, 0:256])
            nc.scalar.copy(s_s[:, 256:512], psm[:, 512:768])

            # M2 = L S: row-block m (= lhsT k-chunk for phase 3) per bank
            m2_s = bpool.tile([128, 512], BF16, tag="m2_s")
            for k in range(2):
                for m in range(2):
                    nc.tensor.matmul(
                        psm[:, m * 512:m * 512 + 256],
                        wl[:, 512 + k * 256 + m * 128:512 + k * 256 + m * 128 + 128],
                        s_s[:, k * 256:(k + 1) * 256],
                        start=(k == 0), stop=(k == 1))
            for _ in range(4):
                nc.tensor.matmul(gps[:, 256:512], wz[:, 0:128],
                                 wz[:, 0:256],
                                 start=True, stop=True, skip_group_check=True)
            nc.vector.tensor_copy(m2_s[:, 0:256], psm[:, 0:256])
            nc.scalar.copy(m2_s[:, 256:512], psm[:, 512:768])

            # ---- phase 3: delta^T = M2^T X^T + t2 (bf16), stream out
            # pz spans 2 PSUM banks; 2 col-blocks of 512 per drain/store
            z_s = [[bpool.tile([128, 1024], BF16, tag=f"z{m}_{q}",
                               name=f"z{m}_{q}") for q in range(4)]
                   for m in range(2)]
            for q in range(4):
                h, p2 = q // 2, q % 2
                for m in range(2):
                    pz = psz.tile([128, 1024], F32, tag="z")
                    for j in range(2):
                        co = (p2 * 2 + j) * 512
                        nc.tensor.matmul(
                            pz[:, j * 512:(j + 1) * 512],
                            m2_s[:, m * 128:(m + 1) * 128],
                            xt_t[0][h][:, co:co + 512],
                            start=True, stop=False)
                        nc.tensor.matmul(
                            pz[:, j * 512:(j + 1) * 512],
                            m2_s[:, 256 + m * 128:256 + (m + 1) * 128],
                            xt_t[1][h][:, co:co + 512],
                            start=False, stop=True)
                    dst = z_s[m][q]
                    if q == 3:
                        # final quarter: split drains for minimum latency
                        nc.vector.tensor_scalar_add(dst[:, 0:512],
                                                    pz[:, 0:512],
                                                    t2c[:, m:m + 1])
                        nc.scalar.activation(
                            dst[:, 512:1024], pz[:, 512:1024],
                            mybir.ActivationFunctionType.Identity,
                            bias=t2c[:, m:m + 1])
                    elif (q * 2 + m) % 2 == 0:
                        nc.vector.tensor_scalar_add(dst[:], pz[:],
                                                    t2c[:, m:m + 1])
                    else:
                        nc.scalar.activation(
                            dst[:], pz[:],
                            mybir.ActivationFunctionType.Identity,
                            bias=t2c[:, m:m + 1])
                    if q == 3:
                        # final stores split across both rings for a short tail
                        nc.scalar.dma_start(
                            dt_d[m * 128:(m + 1) * 128,
                                 q * 1024:q * 1024 + 512], dst[:, 0:512])
                        nc.sync.dma_start(
                            dt_d[m * 128:(m + 1) * 128,
                                 q * 1024 + 512:(q + 1) * 1024],
                            dst[:, 512:1024])
                    else:
                        nc.sync.dma_start(
                            dt_d[m * 128:(m + 1) * 128,
                                 q * 1024:(q + 1) * 1024],
                            dst[:])

    nc.compile()
    return nc


def _get_nc(mode=DEFAULT_MODE):
    key = ("nc", mode)
    if key not in _CACHE:
        _CACHE[key] = _build_nc(mode)
    return _CACHE[key]


def _fold_params(w_g, b_g, w_theta, b_theta, w_phi, b_phi, w_W, b_W,
                 bn_gamma, bn_beta, bn_mean, bn_var):
    f32 = np.float32
    bf = ml_dtypes.bfloat16
    s = (bn_gamma / np.sqrt(bn_var + BN_EPS)).astype(f32)
    t2 = ((b_W - bn_mean) * s + bn_beta).astype(f32)
    L = (np.asarray(w_theta, f32) @ np.asarray(w_phi, f32).T).astype(f32)
    R = (np.asarray(w_g, f32) @ (np.asarray(w_W, f32) * s[None, :]) / N).astype(f32)
    # wl[:, :512][p, k*256+j] = R[k*128+p, j]; wl[:, 512:] likewise for L^T
    pack = lambda M: M.reshape(2, 128, 256).transpose(1, 0, 2).reshape(128, 512)
    wl = np.ascontiguousarray(
        np.concatenate([pack(R), pack(np.ascontiguousarray(L.T))], axis=1)
        .astype(bf))
    t2c = np.ascontiguousarray(t2.reshape(2, 128).T, dtype=f32)
    return wl, t2c


def _reference_fallback(x, w_g, b_g, w_theta, b_theta, w_phi, b_phi, w_W, b_W,
                        bn_gamma, bn_beta, bn_mean, bn_var):
    b, h, w, c = x.shape
    n = h * w
    xf = x.reshape(b, n, c).astype(np.float32)
    g_x = xf @ w_g + b_g
    theta_x = xf @ w_theta + b_theta
    phi_x = xf @ w_phi + b_phi
    a = np.einsum("bnd,bne->bde", phi_x, g_x) / n
    y = theta_x @ a
    w_y = y @ w_W + b_W
    w_y = bn_gamma * (w_y - bn_mean) / np.sqrt(bn_var + BN_EPS) + bn_beta
    return (w_y.reshape(b, h, w, c) + x).astype(np.float32)


def run_sharded(x, folded, mode=DEFAULT_MODE, trace=False):
    from concourse.bass_utils import run_bass_kernel_spmd

    nc = _get_nc(mode)
    wl, t2c = folded
    bf = ml_dtypes.bfloat16
    f8 = ml_dtypes.float8_e4m3
    xf = np.asarray(x, dtype=np.float32).reshape(B, N, C)
    # x8[p, t*256+c] = x[t*128+p, c]
    x8 = np.ascontiguousarray(
        np.clip(xf, -240.0, 240.0).reshape(B, 32, 128, 256)
        .transpose(0, 2, 1, 3).reshape(B, 128, 8192).astype(f8))
    xt = np.ascontiguousarray(xf.transpose(0, 2, 1).astype(bf))
    in_maps = [
        {"x8": x8[i], "xt": xt[i], "wl": wl, "t2c": t2c}
        for i in range(B)
    ]
    res = run_bass_kernel_spmd(nc, in_maps, list(range(B)), trace=trace)
    z = xf + np.stack(
        [np.asarray(res.results[i]["dt"], np.float32).T for i in range(B)],
        axis=0)
    return np.ascontiguousarray(z.reshape(B, H, W, C)), res


def kernel(x, w_g, b_g, w_theta, b_theta, w_phi, b_phi, w_W, b_W,
           bn_gamma, bn_beta, bn_mean, bn_var):
    args = dict(w_g=np.asarray(w_g), b_g=np.asarray(b_g),
                w_theta=np.asarray(w_theta), b_theta=np.asarray(b_theta),
                w_phi=np.asarray(w_phi), b_phi=np.asarray(b_phi),
                w_W=np.asarray(w_W), b_W=np.asarray(b_W),
                bn_gamma=np.asarray(bn_gamma), bn_beta=np.asarray(bn_beta),
                bn_mean=np.asarray(bn_mean), bn_var=np.asarray(bn_var))
    x = np.asarray(x)
    # the device path folds the (zero) projection biases away; anything else
    # (never produced by setup_inputs) gets the exact host fallback
    if (np.any(args["b_g"]) or np.any(args["b_theta"]) or np.any(args["b_phi"])
            or x.shape != (B, H, W, C)):
        return _reference_fallback(x, **{k: v for k, v in args.items()})
    folded = _fold_params(**args)
    z, _ = run_sharded(x, folded)
    return z
